# revision 13
# baseline (speedup 1.0000x reference)
"""Trainium2 Bass kernel for MBart GQA attention.

Problem: B=2, T=2048, E=1024, 16 q-heads, 4 kv-heads, head_dim 64.
Sharding: 8 cores = 2 batches x 4 kv-heads (tensor-parallel over head
groups). Each core computes, for its (batch b, kv-head k):
  - q/k/v projections for its 4 q-heads (q channels k*256:(k+1)*256,
    k/v channels k*64:(k+1)*64), with q pre-scaled by D**-0.5,
  - attention in transposed layout: s^T[tk,tq] = (k_tile)^T-matmuls,
    exp on ScalarE, then out^T = [1|v]^T @ e^T so row 0 of the AV
    accumulator is the softmax denominator,
  - normalization (reciprocal + partition-broadcast + multiply),
  - its partial out-projection  ctx_k @ Wo[:, k*256:(k+1)*256].T.
Host sums the 4 partials per batch and adds bo.

All matmuls bf16 inputs with fp32 PSUM accumulation.
"""

import os
import sys

for _p in ("/opt/trn_rl_repo", "/root/.axon_site/_ro/trn_rl_repo"):
    if os.path.isdir(_p) and _p not in sys.path:
        sys.path.insert(0, _p)

import numpy as np
import ml_dtypes

import concourse.bass as bass
import concourse.mybir as mybir
import concourse.tile as tile
from concourse import bacc
from concourse.bass_utils import run_bass_kernel_spmd

B, T, E = 2, 2048, 1024
H, KVH = 16, 4
D = E // H            # 64
G = H // KVH          # 4 q-heads per kv-head (= per core)
SCALE = D ** -0.5
NCORES = 8

BF16 = mybir.dt.bfloat16
F32 = mybir.dt.float32
NPBF16 = ml_dtypes.bfloat16

ROW_PACK = True  # pack two K=64 score matmuls into the 128x128 PE array


def build_nc(t=T):
    """Build the per-core Bass program (SPMD: same program, per-core data)."""
    assert t % 128 == 0
    ch = min(512, t)          # free-dim chunk for matmuls / psum banks
    ntqc = t // ch            # number of T chunks
    tkt = t // 128            # number of 128-row key tiles
    ne = E // 128             # 8 contraction tiles for projections

    nc = bacc.Bacc(None, target_bir_lowering=False)

    xT_d = nc.declare_dram_parameter("xT", [E, t], BF16, isOutput=False)
    wq_d = nc.declare_dram_parameter("wq", [128, ne, G * D], BF16, isOutput=False)
    wkv_d = nc.declare_dram_parameter("wkv", [128, ne, 2 * D], BF16, isOutput=False)
    wo_d = nc.declare_dram_parameter("wo", [128, 2, E], BF16, isOutput=False)
    bq_d = nc.declare_dram_parameter("bq", [128, 2], F32, isOutput=False)
    bkv_d = nc.declare_dram_parameter("bkv", [128, 1], F32, isOutput=False)
    id_d = nc.declare_dram_parameter("ident", [64, 64], BF16, isOutput=False)
    y_d = nc.declare_dram_parameter("y", [t, E], F32, isOutput=True)

    with tile.TileContext(nc) as tc:
        with (
            tc.tile_pool(name="const", bufs=1) as const,
            tc.tile_pool(name="work", bufs=2) as work,
        ):
            # ---- static SBUF tensors ----
            xT_sb = const.tile([128, ne, t], BF16)
            wq_sb = const.tile([128, ne, G * D], BF16)
            wkv_sb = const.tile([128, ne, 2 * D], BF16)
            wo_sb = const.tile([128, 2, E], BF16)
            bq_sb = const.tile([128, 2], F32)
            bkv_sb = const.tile([128, 1], F32)
            id_sb = const.tile([64, 64], BF16)
            zb_sb = const.tile([128, 1], F32)        # zero bias for Exp
            on_sb = const.tile([1, 1 + D], F32)      # ones row for bcast mm
            qTd_sb = const.tile([128, G, t], BF16)   # q^T per head, dup halves
            kT2_sb = const.tile([128, t], BF16)      # k^T dup in both halves
            vT_sb = const.tile([64, t], BF16)        # v^T at partitions 0-63
            kvn_sb = const.tile([128, t], BF16)      # k^T / v^T proj staging
            va_sb = const.tile([128, tkt, 1 + D], BF16)  # [1|v] per tk tile
            cT_sb = const.tile([128, 2, t], BF16)    # ctx^T (4 heads = 256 ch)

            nc.gpsimd.dma_start(xT_sb[:], xT_d[:].rearrange("(e p) t -> p e t", p=128))
            nc.gpsimd.dma_start(wq_sb[:], wq_d[:])
            nc.gpsimd.dma_start(wkv_sb[:], wkv_d[:])
            nc.gpsimd.dma_start(wo_sb[:], wo_d[:])
            nc.gpsimd.dma_start(bq_sb[:], bq_d[:])
            nc.gpsimd.dma_start(bkv_sb[:], bkv_d[:])
            nc.gpsimd.dma_start(id_sb[:], id_d[:])
            nc.gpsimd.memset(zb_sb[:], 0.0)
            nc.gpsimd.memset(va_sb[:, :, 0], 1.0)
            nc.gpsimd.memset(on_sb[:], 1.0)

            # ---- projections: q^T [256,t], kv^T [128,t] (E-contraction) ----
            with tc.tile_pool(name="psum_proj", bufs=2, space="PSUM") as pp:
                for c in range(ntqc):
                    cs = slice(c * ch, (c + 1) * ch)
                    for w in range(3):
                        ps = pp.tile([128, ch], F32, tag="pp")
                        for e in range(ne):
                            lhsT = (
                                wq_sb[:, e, w * 128:(w + 1) * 128]
                                if w < 2
                                else wkv_sb[:, e, :]
                            )
                            nc.tensor.matmul(
                                ps[:],
                                lhsT,
                                xT_sb[:, e, cs],
                                start=(e == 0),
                                stop=(e == ne - 1),
                            )
                        ident_f = mybir.ActivationFunctionType.Identity
                        if w < 2:
                            # heads 2w (rows 0-63) and 2w+1 (rows 64-127)
                            nc.scalar.activation(
                                qTd_sb[0:64, 2 * w, cs], ps[0:64, :],
                                ident_f, bias=bq_sb[0:64, w:w + 1],
                            )
                            nc.scalar.activation(
                                qTd_sb[64:128, 2 * w + 1, cs], ps[64:128, :],
                                ident_f, bias=bq_sb[64:128, w:w + 1],
                            )
                        else:
                            nc.scalar.activation(
                                kvn_sb[0:64, cs], ps[0:64, :],
                                ident_f, bias=bkv_sb[0:64, :],
                            )
                            nc.scalar.activation(
                                kvn_sb[64:128, cs], ps[64:128, :],
                                ident_f, bias=bkv_sb[64:128, :],
                            )

                # duplicate q per head into both partition halves (row tiling
                # tile T8 reads both operands from partitions 64-127)
                nc.gpsimd.dma_start(qTd_sb[64:128, 0, :], qTd_sb[0:64, 0, :])
                nc.gpsimd.dma_start(qTd_sb[0:64, 1, :], qTd_sb[64:128, 1, :])
                nc.gpsimd.dma_start(qTd_sb[64:128, 2, :], qTd_sb[0:64, 2, :])
                nc.gpsimd.dma_start(qTd_sb[0:64, 3, :], qTd_sb[64:128, 3, :])
                nc.gpsimd.dma_start(kT2_sb[0:64, :], kvn_sb[0:64, :])
                nc.gpsimd.dma_start(kT2_sb[64:128, :], kvn_sb[0:64, :])
                nc.gpsimd.dma_start(vT_sb[:, :], kvn_sb[64:128, :])

                # transpose v^T [64,t] -> v [t,64] into va_sb[:, i, 1:65]
                for i in range(tkt):
                    tp = pp.tile([128, 64], BF16, tag="tp")
                    nc.tensor.transpose(
                        tp[:], vT_sb[:, i * 128:(i + 1) * 128], id_sb[:]
                    )
                    nc.vector.tensor_copy(va_sb[:, i, 1:1 + 64], tp[:])

            # ---- attention + out-projection ----
            psum_attn_cm = tc.tile_pool(name="psum_attn", bufs=1, space="PSUM")
            psum_attn = psum_attn_cm.__enter__()
            for c in range(ntqc):
                cs = slice(c * ch, (c + 1) * ch)
                for h in range(G):
                    sT = work.tile([128, tkt * ch], F32, tag="sT")
                    eT = work.tile([128, tkt * ch], BF16, tag="eT")
                    # scores^T: s[tk, tq] for each 128-row key tile
                    if ROW_PACK:
                        for p in range(tkt // 2):
                            psA = psum_attn.tile([128, ch], F32, tag="sc", bufs=4)
                            psB = psum_attn.tile([128, ch], F32, tag="sc", bufs=4)
                            nc.tensor.matmul(
                                psA[:],
                                kT2_sb[0:64, (2 * p) * 128:(2 * p + 1) * 128],
                                qTd_sb[0:64, h, cs],
                                start=True, stop=True,
                                tile_position=(0, 0),
                            )
                            nc.tensor.matmul(
                                psB[:],
                                kT2_sb[64:128, (2 * p + 1) * 128:(2 * p + 2) * 128],
                                qTd_sb[64:128, h, cs],
                                start=True, stop=True,
                                tile_position=(64, 0),
                            )
                            nc.vector.tensor_copy(
                                sT[:, (2 * p) * ch:(2 * p + 1) * ch], psA[:]
                            )
                            nc.vector.tensor_copy(
                                sT[:, (2 * p + 1) * ch:(2 * p + 2) * ch], psB[:]
                            )
                    else:
                        for p in range(tkt):
                            psA = psum_attn.tile([128, ch], F32, tag="sc", bufs=4)
                            nc.tensor.matmul(
                                psA[:],
                                kT2_sb[0:64, p * 128:(p + 1) * 128],
                                qTd_sb[0:64, h, cs],
                                start=True, stop=True,
                            )
                            nc.vector.tensor_copy(
                                sT[:, p * ch:(p + 1) * ch], psA[:]
                            )

                    # exp over the whole [128, tkt*ch] block in one ACT op
                    nc.scalar.activation(
                        eT[:], sT[:], mybir.ActivationFunctionType.Exp,
                        bias=zb_sb[:],
                    )

                    # out^T accumulate: [1|v]^T @ e^T -> [65, ch]
                    po = psum_attn.tile([1 + D, ch], F32, tag="av", bufs=2)
                    for p in range(tkt):
                        nc.tensor.matmul(
                            po[:],
                            va_sb[:, p, :],
                            eT[:, p * ch:(p + 1) * ch],
                            start=(p == 0),
                            stop=(p == tkt - 1),
                        )

                    # normalize: rows 1-64 divided by row 0 (softmax denom)
                    recip = work.tile([1, ch], F32, tag="recip")
                    nc.vector.reciprocal(recip[:], po[0:1, :])
                    # broadcast recip across partitions: ones[1,65]^T @ recip
                    bc = psum_attn.tile([1 + D, ch], F32, tag="sc", bufs=4)
                    nc.tensor.matmul(bc[:], on_sb[:], recip[:],
                                     start=True, stop=True)
                    bc_sb = work.tile([1 + D, ch], F32, tag="bc_sb")
                    nc.vector.tensor_copy(bc_sb[:], bc[:])
                    cstg = work.tile([1 + D, ch], BF16, tag="cstg")
                    nc.vector.tensor_mul(cstg[:], po[:], bc_sb[:])
                    nc.gpsimd.dma_start(
                        cT_sb[(h % 2) * 64:(h % 2) * 64 + 64, h // 2, cs],
                        cstg[1:1 + 64, :],
                    )

                # out-projection for this T chunk (all 4 heads done)
                for tqt in range(ch // 128):
                    tq0 = c * ch + tqt * 128
                    for nh in range(E // 512):
                        py = psum_attn.tile([128, 512], F32, tag="yp", bufs=2)
                        for ct in range(2):
                            nc.tensor.matmul(
                                py[:],
                                cT_sb[:, ct, tq0:tq0 + 128],
                                wo_sb[:, ct, nh * 512:(nh + 1) * 512],
                                start=(ct == 0),
                                stop=(ct == 1),
                            )
                        ysb = work.tile([128, 512], F32, tag="ysb")
                        nc.vector.tensor_copy(ysb[:], py[:])
                        nc.sync.dma_start(
                            y_d[tq0:tq0 + 128, nh * 512:(nh + 1) * 512], ysb[:]
                        )
            psum_attn_cm.__exit__(None, None, None)

    if hasattr(nc, "compile"):
        nc.compile()
    return nc


def shard_inputs(hidden_states, Wq, bq, Wk, bk, Wv, bv, Wo, bo, t=T):
    """Host-side sharding: returns in_maps for the 8 cores."""
    f32 = np.float32
    x = np.asarray(hidden_states, f32)
    Wq = np.asarray(Wq, f32) * SCALE
    bq = np.asarray(bq, f32) * SCALE
    ident = np.eye(64, dtype=NPBF16)
    ne = E // 128

    in_maps = []
    for cid in range(NCORES):
        b, k = cid // (NCORES // B), cid % (NCORES // B)
        qsl = slice(k * G * D, (k + 1) * G * D)
        ksl = slice(k * D, (k + 1) * D)
        xT = np.ascontiguousarray(x[b, :t].T).astype(NPBF16)          # [E,t]
        wq_l = np.ascontiguousarray(Wq[qsl].T).reshape(ne, 128, G * D)
        wq_l = np.ascontiguousarray(wq_l.transpose(1, 0, 2)).astype(NPBF16)
        wkv = np.concatenate([np.asarray(Wk, f32)[ksl], np.asarray(Wv, f32)[ksl]], 0)
        wkv_l = np.ascontiguousarray(wkv.T).reshape(ne, 128, 2 * D)
        wkv_l = np.ascontiguousarray(wkv_l.transpose(1, 0, 2)).astype(NPBF16)
        wo_l = np.ascontiguousarray(np.asarray(Wo, f32)[:, qsl].T)    # [256,E]
        wo_l = np.ascontiguousarray(
            wo_l.reshape(2, 128, E).transpose(1, 0, 2)
        ).astype(NPBF16)
        bq_l = np.ascontiguousarray(bq[qsl].reshape(2, 128).T).astype(f32)
        bkv_l = np.concatenate(
            [np.asarray(bk, f32)[ksl], np.asarray(bv, f32)[ksl]]
        ).reshape(128, 1).astype(f32)
        in_maps.append({
            "xT": xT, "wq": wq_l, "wkv": wkv_l, "wo": wo_l,
            "bq": bq_l, "bkv": bkv_l, "ident": ident,
        })
    return in_maps


def kernel(**inputs):
    nc = build_nc(T)
    in_maps = shard_inputs(**inputs)
    res = run_bass_kernel_spmd(nc, in_maps, list(range(NCORES)))
    bo = np.asarray(inputs["bo"], np.float32)
    out = np.empty((B, T, E), np.float32)
    for b in range(B):
        acc = np.zeros((T, E), np.float32)
        for k in range(NCORES // B):
            acc += np.asarray(res.results[b * (NCORES // B) + k]["y"], np.float32)
        out[b] = acc + bo
    return out
